# revision 2
# baseline (speedup 1.0000x reference)
"""KMeans assignment kernel for Trainium2 (8 NeuronCores, data-parallel over rows).

argmin_k ||x - c_k||^2  ==  argmax_k (x . c_k - ||c_k||^2 / 2)

Per core (8192 rows):
  - setup: load centers, square+reduce for |c|^2, PE-transpose to cT[d, k],
    build bias row (-|c|^2/2) on one partition.
  - main loop over 64 row-tiles of 128 rows:
      PE-transpose x tile -> xT[d, rows]; for each 512-wide k-chunk:
      rank-1 bias matmul + 4 accumulating matmuls -> PSUM = scores chunk;
      ACT copies PSUM -> SBUF; DVE max8 + max_index8 -> argmax per row.
  - gather per-tile indices into a staging tile, single DMA out.
"""

import numpy as np

N, D, K = 65536, 512, 4096
NCORES = 8
NLOC = N // NCORES            # 8192 rows per core
P = 128
N_ROW_TILES = NLOC // P       # 64
KC = 512                      # k-chunk width (one PSUM bank)
N_K_CHUNKS = K // KC          # 8
N_D_CHUNKS = D // P           # 4

# 'fp32' (exact, 4 cyc/col) or 'f32r' (1 cyc/col, ~12-bit mantissa rounding)
MATMUL_MODE = "fp32"

_cache = {}


def _build(mode):
    import concourse.bacc as bacc
    import concourse.mybir as mybir
    from concourse.tile import TileContext
    from concourse.masks import make_identity

    f32 = mybir.dt.float32
    f32r = mybir.dt.float32r
    mm_dt = f32r if mode == "f32r" else f32

    nc = bacc.Bacc("TRN2", target_bir_lowering=False)

    x_d = nc.dram_tensor("x_loc", [NLOC, D], f32, kind="ExternalInput")
    c_d = nc.dram_tensor("center", [K, D], f32, kind="ExternalInput")
    o_d = nc.dram_tensor("idx_out", [NLOC], mybir.dt.int32, kind="ExternalOutput")

    with TileContext(nc) as tc:
        with tc.tile_pool(name="const", bufs=1) as const_pool, \
             tc.tile_pool(name="setup", bufs=3) as setup_pool, \
             tc.tile_pool(name="xio", bufs=3) as x_pool, \
             tc.tile_pool(name="xTp", bufs=2) as xT_pool, \
             tc.tile_pool(name="scp", bufs=2) as sc_pool, \
             tc.tile_pool(name="small", bufs=2) as small_pool, \
             tc.tile_pool(name="ptrp", bufs=2, space="PSUM") as psum_tr_pool, \
             tc.tile_pool(name="pmmp", bufs=4, space="PSUM") as psum_mm_pool:

            ident = const_pool.tile([P, P], f32)
            make_identity(nc, ident)

            # ---- setup: centers ----
            cT = const_pool.tile([P, N_D_CHUNKS * K], mm_dt)  # [d, dc*K + k]
            csq_cols = const_pool.tile([P, 32], f32)          # csq for center 128*t+p at [p, t]
            for t in range(K // P):
                ct = setup_pool.tile([P, D], f32, name="ct", tag="ct")
                nc.sync.dma_start(ct, c_d[t * P:(t + 1) * P, :])
                sq = setup_pool.tile([P, D], f32, name="sq", tag="sq")
                nc.scalar.activation(sq, ct, mybir.ActivationFunctionType.Square)
                nc.vector.reduce_sum(csq_cols[:, t:t + 1], sq, axis=mybir.AxisListType.X)
                for dc in range(N_D_CHUNKS):
                    ptr = psum_tr_pool.tile([P, P], f32, name="ptr", tag="ptr")
                    nc.tensor.transpose(ptr, ct[:, dc * P:(dc + 1) * P], ident)
                    nc.scalar.copy(cT[:, dc * K + t * P: dc * K + (t + 1) * P], ptr)

            # ---- setup: bias row = -csq/2 laid out [1, K] on partition 0 ----
            nsq = const_pool.tile([P, 32], f32)
            nc.vector.tensor_scalar_mul(nsq, csq_cols, -0.5)
            ptr2 = psum_tr_pool.tile([P, P], f32, name="ptr2", tag="ptr")
            nc.tensor.transpose(ptr2[:32, :], nsq, ident)
            csqT = const_pool.tile([32, P], f32)
            nc.scalar.copy(csqT, ptr2[:32, :])
            bias_row = const_pool.tile([1, K], f32)
            nc.sync.dma_start(bias_row.rearrange("a (t f) -> a t f", t=32), csqT)

            ones_row = const_pool.tile([1, P], f32)
            nc.vector.memset(ones_row, 1.0)

            if mode == "f32r":
                bias_row_mm = const_pool.tile([1, K], f32r)
                nc.vector.tensor_copy(bias_row_mm, bias_row)
                ones_row_mm = const_pool.tile([1, P], f32r)
                nc.vector.tensor_copy(ones_row_mm, ones_row)
            else:
                bias_row_mm = bias_row
                ones_row_mm = ones_row

            stage = const_pool.tile([P, N_ROW_TILES], mybir.dt.int32)

            # ---- main loop ----
            for t in range(N_ROW_TILES):
                xt = x_pool.tile([P, D], f32, name="xt", tag="xt")
                nc.sync.dma_start(xt, x_d[t * P:(t + 1) * P, :])
                xT = xT_pool.tile([P, D], mm_dt, name="xT", tag="xT")
                for dc in range(N_D_CHUNKS):
                    ptr = psum_tr_pool.tile([P, P], f32, name="ptrm", tag="ptr")
                    nc.tensor.transpose(ptr, xt[:, dc * P:(dc + 1) * P], ident)
                    nc.scalar.copy(xT[:, dc * P:(dc + 1) * P], ptr)

                sc = sc_pool.tile([P, K], f32, name="sc", tag="sc")
                for j in range(N_K_CHUNKS):
                    ps = psum_mm_pool.tile([P, KC], f32, name="ps", tag="mm")
                    # bias first: scores = ones^T ones_row? no: rank-1 (-csq/2) broadcast
                    nc.tensor.matmul(ps, ones_row_mm, bias_row_mm[:, j * KC:(j + 1) * KC],
                                     start=True, stop=False)
                    for dc in range(N_D_CHUNKS):
                        nc.tensor.matmul(
                            ps,
                            xT[:, dc * P:(dc + 1) * P],
                            cT[:, dc * K + j * KC: dc * K + (j + 1) * KC],
                            start=False, stop=(dc == N_D_CHUNKS - 1),
                        )
                    nc.scalar.copy(sc[:, j * KC:(j + 1) * KC], ps)

                m8 = small_pool.tile([P, 8], f32, name="m8", tag="m8")
                i8 = small_pool.tile([P, 8], mybir.dt.uint32, name="i8", tag="i8")
                nc.vector.max(out=m8, in_=sc)
                nc.vector.max_index(i8, m8, sc)
                nc.vector.tensor_copy(stage[:, t:t + 1], i8[:, 0:1])

            nc.sync.dma_start(o_d[:].rearrange("(t p) -> p t", p=P), stage)

    nc.compile()
    return nc


def _get_nc(mode):
    if mode not in _cache:
        _cache[mode] = _build(mode)
    return _cache[mode]


def kernel(x, center):
    from concourse.bass_utils import run_bass_kernel_spmd

    x = np.ascontiguousarray(np.asarray(x), dtype=np.float32)
    center = np.ascontiguousarray(np.asarray(center), dtype=np.float32)
    assert x.shape == (N, D) and center.shape == (K, D)

    nc = _get_nc(MATMUL_MODE)
    in_maps = [
        {"x_loc": x[i * NLOC:(i + 1) * NLOC], "center": center}
        for i in range(NCORES)
    ]
    res = run_bass_kernel_spmd(nc, in_maps, core_ids=list(range(NCORES)))
    return np.concatenate([r["idx_out"] for r in res.results]).astype(np.int32)
